# revision 33
# baseline (speedup 1.0000x reference)
"""TRN2 Bass kernel for nn_CNNDSTv2_batch: out = mobius16(zeta16(M[:,0]) * zeta16(M[:,1])).

Math: the 16-bit superset-zeta factorizes as Z = A8 @ X @ A8^T on the 256x256
view X[hi_byte, lo_byte]; A8 = [[A7, A7], [0, A7]] block-triangular, so each
8-bit stage is 3 accumulating 128x128 matmuls reusing one stationary. Each
two-sided transform runs as [stage, transpose, stage] and yields the transposed
result; chaining zeta -> multiply -> mobius lands back in natural layout.

Precision (modeled in numpy against f64, gate 2e-2, model says 3.9e-3):
- zeta path runs fully in bf16 (inputs are positive and the zeta->product->
  mobius composition is a positive map, so input/mid roundings stay relative);
- q (the commonality product feeding Mobius) keeps a 2-term f32r hi/lo split -
  rounding q is amplified ~100x by Mobius cancellation and dominates the
  error budget if single-rounded;
- the mobius mid-plane u is single-rounded f32r; the output is written bf16
  (final rounding, unamplified).

Perf: bf16 halves input DMA, y-plane copy bytes (2x DVE rate) and transpose
cost (1.0 cyc/row), and bf16 transpose PSUM tiles are 1 bank. walrus
enable-ldw-opt stays OFF (it miscompiles bf16/fp32 is_transpose); the tile
layer's own hoisted InstLdweights split gives overlapped weight loads anyway.
Emission is a flat skewed software pipeline (one pair per slot, 17 stage
positions) so the PE never waits on a same-pair dependency chain and holds
its 2.4 GHz p-state. Per-plane copies sit on Activation/DVE; the Pool engine
(slow, ~1.35us per 512-wide op, no PSUM access) carries three butterfly
pre-adds per pair, each replacing a 512-row matmul on the PE.

Sharding: pure data parallel, batch 512 -> 64 per core across 8 cores.
"""
import sys
import os
import functools

sys.path.insert(0, "/opt/trn_rl_repo")
import numpy as np

BATCH = 512
L = 65536
NCORES = 8
BPC = BATCH // NCORES          # 64 batch elems per core
PAIRS = BPC // 2               # 2 elems per pipeline iteration


def _pc(v):
    return bin(v).count("1")


def _constants():
    k = np.arange(128)
    sup = (k[:, None] & k[None, :]) == k[None, :]          # sup[k,m] = k superset of m
    AT7 = sup.astype(np.float32)                           # lhsT for A7 @ x
    pc = np.array([_pc(i) for i in range(128)])
    sign = (-1.0) ** (pc[:, None] - pc[None, :])
    BT7 = (sup * sign).astype(np.float32)                  # lhsT for B7 @ x
    return AT7, BT7


def _build():
    import concourse.bacc as bacc
    import concourse.tile as tile
    import concourse.mybir as mybir

    # NOTE: walrus --enable-ldw-opt stays OFF: it miscompiles bf16 (and fp32)
    # is_transpose matmuls (verified: transposed planes come out as garbage).
    # The tile layer already pre-splits 2-byte-dtype matmuls into hoisted
    # InstLdweights + matmul, so bf16 weight loads overlap anyway; only the
    # f32r mobius matmuls pay a serial self-load.
    dt = mybir.dt
    F32, F32R, BF16 = dt.float32, dt.float32r, dt.bfloat16

    nc = bacc.Bacc("TRN2", target_bir_lowering=False, debug=False)

    # HBM layout (host pre-permuted, all DMAs contiguous):
    # Mi[pair, ch, p(=bits14..8), (I=bit15, b, J=bit7, l=bits6..0)] in bf16
    # (I outermost so the I=0/1 halves are contiguous 512-wide slices)
    Mi = nc.dram_tensor("Mi", [PAIRS, 2, 128, 1024], BF16, kind="ExternalInput").ap()
    # Cb = [AT7 | Id] bf16 (exact 0/1), Cr = [BT7 | -BT7 | Id | AT7] f32r
    Cb_d = nc.dram_tensor("Cb", [128, 256], BF16, kind="ExternalInput").ap()
    Cr_d = nc.dram_tensor("Cr", [128, 512], F32R, kind="ExternalInput").ap()
    # O[pair, p, (I''=bit15, b, J=bit7, l=bits6..0)] bf16 - host unscrambles
    O = nc.dram_tensor("O", [PAIRS, 128, 1024], BF16, kind="ExternalOutput").ap()

    with tile.TileContext(nc) as tc:
        with tc.tile_pool(name="const", bufs=1) as cp, \
             tc.tile_pool(name="sbuf", bufs=2) as sb, \
             tc.tile_pool(name="psA", bufs=3, space="PSUM") as psA:
            Cb = cp.tile([128, 256], BF16, tag="Cb")
            nc.sync.dma_start(Cb[:], Cb_d)
            Cr = cp.tile([128, 512], F32R, tag="Cr")
            nc.sync.dma_start(Cr[:], Cr_d)
            ATb = Cb[:, 0:128]
            Idb = Cb[:, 128:256]
            BT = Cr[:, 0:128]
            nBT = Cr[:, 128:256]
            IdR = Cr[:, 256:384]
            ATr = Cr[:, 384:512]

            def mm(out_ap, lhsT, rhs, start, stop):
                nc.tensor.matmul(out_ap, lhsT, rhs, start=start, stop=stop)

            def transpose_plane(dst, src, Id):
                """dst[:, Jd*512 + b*256 + K*128 +: 128] =
                   src[:, K*512 + b*256 + Jd*128 +: 128].T  for Jd,b,K in {0,1}.
                One start/stop group per 512-wide half."""
                for Jd in (0, 1):
                    k = 0
                    for b in (0, 1):
                        for K in (0, 1):
                            nc.tensor.matmul(
                                dst[:, Jd * 512 + b * 256 + K * 128:][:, :128],
                                src[:, K * 512 + b * 256 + Jd * 128:][:, :128],
                                Id, is_transpose=True,
                                start=(k == 0), stop=(k == 3))
                            k += 1

            # --- software-pipelined emission: 2 pairs interleaved ---
            st = {}

            def dma_in(pr, c):
                xin = sb.tile([128, 1024], BF16, tag=f"xin{c}", bufs=5,
                              name=f"xin{c}")
                nc.sync.dma_start(xin[:], Mi[pr, c])
                st[pr, c, "x"] = xin

            def pre_w(pr, c):
                # I-bit butterfly pre-add on the idle Pool engine: the sum of
                # two bf16 values is a <=9-bit mantissa, so bf16 out is ~exact
                x = st[pr, c, "x"]
                w = sb.tile([128, 512], BF16, tag=f"w{c}", name=f"w{c}", bufs=3)
                nc.gpsimd.tensor_add(w[:], x[:, 0:512], x[:, 512:1024])
                st[pr, c, "w"] = w

            def zeta_s1(pr, c):
                # x free layout (I, b, J, l): d1 = A7@x_I1 ; d0 = A7@(x_I0+x_I1)
                x = st[pr, c, "x"]
                y = psA.tile([128, 1024], F32, tag="a", name="y")
                mm(y[:, 512:1024], ATb, x[:, 512:1024], start=True, stop=True)
                mm(y[:, 0:512], ATb, st[pr, c, "w"][:], start=True, stop=True)
                st[pr, c, "y"] = y

            def copy_ys(pr, c):
                # PSUM f32 -> SBUF bf16, single rounding on the zeta path
                y = st[pr, c, "y"]
                ys = sb.tile([128, 1024], BF16, tag=f"ys{c}", name=f"ys{c}", bufs=3)
                nc.scalar.copy(ys[:], y[:])
                st[pr, c, "ys"] = ys

            def trans_y(pr, c):
                yT = psA.tile([128, 1024], BF16, tag="pT", bufs=2, name="yT")
                transpose_plane(yT[:], st[pr, c, "ys"][:], Idb)
                st[pr, c, "yT"] = yT

            def copy_yTs(pr, c):
                yTs = sb.tile([128, 1024], BF16, tag=f"yTs{c}", name=f"yTs{c}", bufs=3)
                nc.vector.tensor_copy(yTs[:], st[pr, c, "yT"][:])
                st[pr, c, "Ts"] = yTs

            def zeta_s2(pr, c):
                # 3-matmul form: a J-bit pre-add would need Pool time the
                # engine doesn't have (it saturated at 5 pre-ops/pair)
                yTs = st[pr, c, "Ts"]
                z = psA.tile([128, 1024], F32, tag="a", name="z")
                mm(z[:, 512:1024], ATb, yTs[:, 512:1024], start=True, stop=True)
                mm(z[:, 0:512], ATb, yTs[:, 0:512], start=True, stop=False)
                mm(z[:, 0:512], ATb, yTs[:, 512:1024], start=False, stop=True)
                if c == 0:
                    z0s = sb.tile([128, 1024], F32, tag="z0s", name="z0s", bufs=3)
                    nc.scalar.copy(z0s[:], z[:])
                    st[pr, "z0s"] = z0s
                else:
                    qf = sb.tile([128, 1024], F32, tag="qf", name="qf", bufs=3)
                    nc.vector.tensor_mul(qf[:], z[:], st[pr, "z0s"][:])
                    # q hi/lo split on the fast engines: Pool is too slow for
                    # this chain (its 3.6us CAST stalled the PE every slot)
                    qh = sb.tile([128, 1024], F32R, tag="qh", name="qh", bufs=3)
                    nc.scalar.copy(qh[:], qf[:])
                    ql = sb.tile([128, 1024], F32R, tag="ql", name="ql", bufs=3)
                    nc.vector.tensor_sub(ql[:], qf[:], qh[:].bitcast(F32))
                    st[pr, "q"] = (qh, ql)

            def mob_s1(pr):
                # like stage(), but all qh-dependent matmuls are issued before
                # the ql-dependent ones: ql is one DVE op behind qh, so this
                # gives the PE ~0.6us of ready work while ql lands
                qh, ql = st[pr, "q"]
                u = psA.tile([128, 1024], F32, tag="a", name="u")
                d0, d1 = u[:, 0:512], u[:, 512:1024]
                mm(d1, BT, qh[:, 512:1024], start=True, stop=False)
                mm(d0, BT, qh[:, 0:512], start=True, stop=False)
                mm(d0, nBT, qh[:, 512:1024], start=False, stop=False)
                mm(d0, BT, ql[:, 0:512], start=False, stop=False)
                mm(d0, nBT, ql[:, 512:1024], start=False, stop=True)
                mm(d1, BT, ql[:, 512:1024], start=False, stop=True)
                st[pr, "u"] = u

            def copy_us(pr):
                us = sb.tile([128, 1024], F32R, tag="us", name="us", bufs=3)
                nc.scalar.copy(us[:], st[pr, "u"][:])
                st[pr, "us"] = us

            def trans_u(pr):
                uT = psA.tile([128, 1024], F32R, tag="a", name="uT")
                transpose_plane(uT[:], st[pr, "us"][:], IdR)
                st[pr, "uT"] = uT

            def copy_uTs(pr):
                uTs = sb.tile([128, 1024], F32R, tag="uTs", name="uTs", bufs=3)
                nc.vector.tensor_copy(uTs[:], st[pr, "uT"][:])
                st[pr, "uTs"] = uTs

            def pre_w3(pr):
                # I-bit Mobius pre-sub: operands are already f32r, so the f32r
                # difference is near-exact (verified benign in the error model)
                uTs = st[pr, "uTs"]
                w3 = sb.tile([128, 512], F32R, tag="w3", name="w3", bufs=3)
                nc.gpsimd.tensor_sub(w3[:], uTs[:, 0:512], uTs[:, 512:1024])
                st[pr, "w3"] = w3

            def mob_s2(pr):
                uTs = st[pr, "uTs"]
                o = psA.tile([128, 1024], F32, tag="a", name="o")
                mm(o[:, 512:1024], BT, uTs[:, 512:1024], start=True, stop=True)
                mm(o[:, 0:512], BT, st[pr, "w3"][:], start=True, stop=True)
                osb = sb.tile([128, 1024], BF16, tag="osb", name="osb")
                nc.vector.tensor_copy(osb[:], o[:])
                nc.sync.dma_start(O[pr], osb[:])

            # Flat skewed software pipeline: pair pr runs stage at position p
            # during slot t = pr + p. Each slot emits one stage of ~18
            # different pairs, later stages first, so every engine's queue
            # interleaves many pairs and per-pair dependency chains never
            # stall the PE (which also keeps it at the 2.4 GHz p-state).
            stages = [
                (0, lambda pr: dma_in(pr, 0)),
                (1, lambda pr: (dma_in(pr, 1), pre_w(pr, 0))),
                (2, lambda pr: pre_w(pr, 1)),
                (3, lambda pr: zeta_s1(pr, 0)),
                (4, lambda pr: (zeta_s1(pr, 1), copy_ys(pr, 0))),
                (5, lambda pr: copy_ys(pr, 1)),
                (6, lambda pr: trans_y(pr, 0)),
                (7, lambda pr: (trans_y(pr, 1), copy_yTs(pr, 0))),
                (8, lambda pr: (copy_yTs(pr, 1), zeta_s2(pr, 0))),
                (9, lambda pr: zeta_s2(pr, 1)),
                (10, lambda pr: mob_s1(pr)),
                (11, lambda pr: copy_us(pr)),
                (13, lambda pr: trans_u(pr)),
                (14, lambda pr: copy_uTs(pr)),
                (15, lambda pr: pre_w3(pr)),
                (16, lambda pr: mob_s2(pr)),
            ]
            stages.sort(key=lambda s: -s[0])
            LAST = stages[0][0]
            for t in range(PAIRS + LAST):
                for pos, fn in stages:
                    pr = t - pos
                    if 0 <= pr < PAIRS:
                        fn(pr)

    nc.compile()
    return nc


@functools.lru_cache(maxsize=1)
def _get_nc():
    return _build()


def _host_in(M):
    """M [512, 2, 65536] f32 -> per-core Mi [PAIRS, 2, 128, 1024] bf16 contiguous.
    index16 = I*2^15 + p*2^8 + J*2^7 + l ; f-order (I, b, J, l)."""
    import ml_dtypes
    M6 = np.asarray(M, dtype=np.float32).reshape(NCORES, PAIRS, 2, 2, 2, 128, 2, 128)
    #                                      core, pair, b,  ch, I,  p,   J,  l
    Mi = np.ascontiguousarray(M6.transpose(0, 1, 3, 5, 4, 2, 6, 7))
    #                                      core, pair, ch, p, I, b, J, l
    return Mi.reshape(NCORES, PAIRS, 2, 128, 1024).astype(ml_dtypes.bfloat16)


def _host_out(Os):
    """Os list of [PAIRS, 128, 1024] bf16 per core -> [512, 65536, 1, 1] f32.
    o f-layout (I'', b, J, l)."""
    O = np.stack(Os).astype(np.float32).reshape(NCORES, PAIRS, 128, 2, 2, 2, 128)
    #                                            core, pair, p, I, b, J, l
    out = np.ascontiguousarray(O.transpose(0, 1, 4, 3, 2, 5, 6))
    #                                      core, pair, b, I, p, J, l
    return out.reshape(BATCH, L, 1, 1)


def _run(M, trace=False):
    import ml_dtypes
    from concourse.bass_utils import run_bass_kernel_spmd
    nc = _get_nc()
    AT7, BT7 = _constants()
    Cb = np.concatenate([AT7, np.eye(128, dtype=np.float32)],
                        axis=1).astype(ml_dtypes.bfloat16)
    Cr = np.concatenate([BT7, -BT7, np.eye(128, dtype=np.float32), AT7], axis=1)
    Mi = _host_in(M)
    in_maps = [{"Mi": Mi[k], "Cb": Cb, "Cr": Cr} for k in range(NCORES)]
    res = run_bass_kernel_spmd(nc, in_maps, list(range(NCORES)), trace=trace)
    out = _host_out([res.results[k]["O"] for k in range(NCORES)])
    return out, res


def kernel(M):
    try:
        out, _ = _run(M, trace=False)
    except Exception:
        # one retry: a cold first execute has been observed to flake
        # (NRT_EXEC_UNIT_UNRECOVERABLE) and recover on rerun
        out, _ = _run(M, trace=False)
    return out
